# revision 1
# baseline (speedup 1.0000x reference)
"""Trainium2 Bass kernel for nn_Encoder_75436805587012 (6-layer dense
transformer encoder: B=2, S=1024, D=1024, H=16, DFF=4096, VFS=2048).

Sharding: 8-way token parallelism. Cores 0-3 take batch 0, cores 4-7 batch 1;
each core owns 256 contiguous tokens of its sequence. Weights are replicated
(streamed from HBM); per-layer K/V are AllGathered within each 4-core batch
group so every core attends over its full sequence.

On-chip layout: activations are feature-major ("fm", [feature, token]),
making every projection transpose-free:
    out_T[fo, tok] = W.T @ h_T     (lhsT = W as stored [fi, fo], rhs = h_T)
V is produced token-major via the dual form (lhsT = h_T token-slice, rhs = W).
Attention computes transposed logits  logits_T[kt, q] = (K head cols).T @ Q_fm
with max-free softmax: exp folds into the PSUM eviction (scale = 1/sqrt(64)),
the attention mask folds into the per-partition exp bias, and the softmax
denominator comes from a ones-augmented column in the A@V matmul.
Feature-axis LayerNorm uses ones-matmul partition reductions on TensorE and a
K=1 ones-outer-product to broadcast per-token stats across partitions.

Matmuls run in float32r (tf32-like: full rate at N>=256, ~1.5e-4 rel error)
with fp32 PSUM accumulation.
"""
import numpy as np

import concourse.bass as bass
import concourse.mybir as mybir
import concourse.tile as tile
from concourse import bacc
from concourse.bass_utils import run_bass_kernel_spmd
from concourse.masks import make_identity

F32 = mybir.dt.float32
F32R = mybir.dt.float32r
AF = mybir.ActivationFunctionType
AX = mybir.AxisListType

L, D, H, DFF, VFS, MAXPOS = 6, 1024, 16, 4096, 2048, 2048
DEPTH = D // H              # 64
B, S = 2, 1024
NCORES, GROUP = 8, 4
TOK = (B * S) // NCORES     # 256 tokens per core
P = 128
KD, KV, KF = D // P, VFS // P, DFF // P     # 8, 16, 32
LN_EPS = 1e-5
SCALE = 1.0 / float(np.sqrt(np.float32(DEPTH)))


def build_graph(n_layers=L, reps=1, dbg=False, inline=False, sim1=False,
                ablate=()):
    """One SPMD program; all 8 cores run it on their own token slice.

    inline=True builds a timing-only variant: all big inputs become NEFF
    const tensors (random data; per-layer weights shared) so per-call IO
    shipping over the axon tunnel is negligible and wall-clock deltas
    reflect device execution time. Numerics are garbage by construction.
    """
    nc = bacc.Bacc(None, target_bir_lowering=False,
                   num_devices=1 if sim1 else NCORES)
    _rng = np.random.default_rng(0)

    def _ext(name, shape, dt_, fill=0.02):
        if not inline:
            return nc.dram_tensor(name, shape, dt_, kind="ExternalInput")
        if fill == "ones":
            data = np.ones(shape, np.float32)
        elif fill == 0.0:
            data = np.zeros(shape, np.float32)
        else:
            data = (_rng.standard_normal(shape) * fill).astype(np.float32)
        hdl = nc.inline_tensor(data, name=name)
        if dt_ == F32R:
            nc.lookup_mls(hdl).dtype = F32R
            hdl = bass.DRamTensorHandle(name, list(data.shape), F32R)
        return hdl
    dbg_t = {}
    if dbg:
        for nm, shape in [("dbg_xT", [VFS, TOK]), ("dbg_emb", [D, TOK]),
                          ("dbg_ln2", [D, TOK]), ("dbg_pos", [D, TOK])]:
            dbg_t[nm] = nc.dram_tensor(nm, shape, F32, kind="ExternalOutput")

    # ---------------- I/O ----------------
    LW = 1 if inline else L    # timing variant shares one layer's weights
    xs = _ext("xs", [TOK, VFS], F32, 1.0)
    post = _ext("post", [D, TOK], F32, 0.5)
    maskc = _ext("maskc", [P, KD], F32, 0.0)
    embw = _ext("embw", [VFS, D], F32R)
    embbc = _ext("embbc", [P, KD], F32, 0.0)
    eg = _ext("eg", [P, 4 * KD], F32, "ones")
    wq = _ext("wq", [LW, D, D], F32R)
    wk = _ext("wk", [LW, D, D], F32R)
    wv = _ext("wv", [LW, D, D], F32R)
    wo = _ext("wo", [LW, D, D], F32R)
    w1 = _ext("w1", [LW, D, DFF], F32R)
    w2 = _ext("w2", [LW, DFF, D], F32R)
    # per-layer small params, packed column tiles; layout in make_in_maps
    bcol = _ext("bcol", [LW, P, 8 * KD], F32, "ones")
    b1col = _ext("b1col", [LW, P, KF], F32, 0.0)
    bvr = _ext("bvr", [LW, 1, D], F32R, 0.0)
    chain = nc.dram_tensor("chain", [1, 1], F32, kind="ExternalInput")
    out = nc.dram_tensor("out", [TOK, D], F32, kind="ExternalOutput")
    chain_out = nc.dram_tensor("chain_out", [1, 1], F32, kind="ExternalOutput")

    rg = [[0, 1, 2, 3], [4, 5, 6, 7]]
    ccs = []
    for r in range(reps):
        for l in range(n_layers):
            kin = nc.dram_tensor(f"cc_k_in_{r}_{l}", [D, TOK], F32R)
            kout = nc.dram_tensor(f"cc_k_out_{r}_{l}", [GROUP * D, TOK], F32R)
            vin = nc.dram_tensor(f"cc_v_in_{r}_{l}", [TOK, D], F32R)
            vout = nc.dram_tensor(f"cc_v_out_{r}_{l}", [GROUP * TOK, D], F32R)
            ccs.append((kin, kout, vin, vout))

    with tile.TileContext(nc) as tc:
        import contextlib
        stack = contextlib.ExitStack()
        stack.enter_context(nc.allow_low_precision(
            reason="fp32r tiles are the matmul compute dtype; fp32 PSUM"))
        const = stack.enter_context(tc.tile_pool(name="const", bufs=1))
        hp = stack.enter_context(tc.tile_pool(name="hp", bufs=1))
        wp = stack.enter_context(tc.tile_pool(name="wp", bufs=4))
        sp = stack.enter_context(tc.tile_pool(name="sp", bufs=3))
        ps = stack.enter_context(tc.tile_pool(name="ps", bufs=8, space="PSUM"))

        # ---------------- constants ----------------
        ident = const.tile([P, P], F32)
        make_identity(nc, ident)
        ones_f = const.tile([P, 1], F32)
        nc.any.memset(ones_f[:], 1.0)
        ones_col = const.tile([P, 1], F32R)
        nc.vector.tensor_copy(ones_col[:], ones_f[:])
        ones_row_f = const.tile([1, P], F32)
        nc.any.memset(ones_row_f[:], 1.0)
        ones_row = const.tile([1, P], F32R)
        nc.vector.tensor_copy(ones_row[:], ones_row_f[:])
        mask_sb = const.tile([P, KD], F32)
        nc.sync.dma_start(mask_sb[:], maskc[:])
        eps_col = const.tile([P, 1], F32)
        nc.any.memset(eps_col[:], LN_EPS)

        def psum(name):
            return ps.tile([P, 2 * TOK], F32, name=name, tag="ps")

        def ln_fm(xt, gb_sb, gcol, bcol_, out_dtype=F32R):
            """LayerNorm over features (partition axis) of KD fm tiles."""
            pst_s = psum("pst_s")
            for i in range(KD):
                nc.tensor.matmul(pst_s[:1, 0:TOK], ones_col[:], xt[i][:],
                                 start=(i == 0), stop=(i == KD - 1))
            pst_s2 = psum("pst_s2")
            for i in range(KD):
                sq = sp.tile([P, TOK], F32R, name="sq", tag="sq", bufs=2)
                nc.scalar.activation(sq[:], xt[i][:], AF.Square)
                nc.tensor.matmul(pst_s2[:1, 0:TOK], ones_col[:], sq[:],
                                 start=(i == 0), stop=(i == KD - 1))
            mu = sp.tile([1, TOK], F32, name="mu", tag="mu", bufs=1)
            nc.scalar.activation(mu[:], pst_s[0:1, 0:TOK], AF.Copy, scale=1.0 / D)
            ex2 = sp.tile([1, TOK], F32, name="ex2", tag="ex2", bufs=1)
            nc.scalar.activation(ex2[:], pst_s2[0:1, 0:TOK], AF.Copy,
                                 scale=1.0 / D)
            mu2 = sp.tile([1, TOK], F32, name="mu2", tag="mu2", bufs=1)
            nc.scalar.activation(mu2[:], mu[:], AF.Square)
            var = sp.tile([1, TOK], F32, name="var", tag="var", bufs=1)
            nc.vector.tensor_sub(var[:], ex2[:], mu2[:])
            sd = sp.tile([1, TOK], F32, name="sd", tag="sd", bufs=1)
            nc.scalar.activation(sd[:], var[:], AF.Sqrt, bias=eps_col[0:1, :])
            a_r = sp.tile([1, TOK], F32R, name="a_r", tag="a_r", bufs=1)
            nc.vector.reciprocal(a_r[:], sd[:])
            nmu = sp.tile([1, TOK], F32, name="nmu", tag="nmu", bufs=1)
            nc.scalar.activation(nmu[:], mu[:], AF.Copy, scale=-1.0)
            c_r = sp.tile([1, TOK], F32R, name="c_r", tag="c_r", bufs=1)
            nc.vector.tensor_mul(c_r[:], nmu[:], a_r[:].bitcast(F32))
            pac_a = psum("pac_a")
            nc.tensor.matmul(pac_a[:, 0:TOK], ones_row[:], a_r[:],
                             start=True, stop=True)
            pac_c = psum("pac_c")
            nc.tensor.matmul(pac_c[:, 0:TOK], ones_row[:], c_r[:],
                             start=True, stop=True)
            outt = []
            for i in range(KD):
                t1 = sp.tile([P, TOK], F32, name="lnt1", tag="lnt1", bufs=2)
                nc.vector.tensor_mul(t1[:], xt[i][:].bitcast(F32), pac_a[:, 0:TOK])
                t2 = sp.tile([P, TOK], F32, name="lnt2", tag="lnt2", bufs=2)
                nc.vector.tensor_add(t2[:], t1[:], pac_c[:, 0:TOK])
                o = hp.tile([P, TOK], out_dtype, name="h", tag="lnout", bufs=10)
                nc.scalar.activation(o[:], t2[:], AF.Identity,
                                     bias=gb_sb[:, bcol_ + i:bcol_ + i + 1],
                                     scale=gb_sb[:, gcol + i:gcol + i + 1])
                outt.append(o)
            return outt

        def proj_fm(w2d, ht, bias_sb, bias_col, func=AF.Identity, alpha=0.0,
                    out_dtype=F32R, n_out=KD, tag="proj", out_bufs=8,
                    col0=0):
            """Mode A: out_T[fo,tok] = W.T @ h_T (+bias, func).
            w2d: DRAM AP [len(ht)*128, >= col0 + n_out*128] (layer-sliced).
            k-outer / m-inner: streams one [128, n_out*128] stripe per k.
            """
            kt = len(ht)
            pss = [psum(f"pp{m}") for m in range(n_out)]
            st0 = None
            for k in range(kt):
                if "now" in ablate and st0 is not None:
                    st = st0
                else:
                    st = wp.tile([P, n_out * P], F32R, name="wst", tag="w",
                                 bufs=4)
                    nc.sync.dma_start(
                        st[:], w2d[k * P:(k + 1) * P, col0:col0 + n_out * P])
                    st0 = st
                for m in range(n_out):
                    nc.tensor.matmul(
                        pss[m][:, 0:TOK], st[:, m * P:(m + 1) * P], ht[k][:],
                        start=(k == 0), stop=(k == kt - 1))
            outs = []
            for m in range(n_out):
                o = hp.tile([P, TOK], out_dtype, name=tag, tag=tag,
                            bufs=out_bufs)
                nc.scalar.activation(
                    o[:], pss[m][:, 0:TOK], func, alpha=alpha,
                    bias=bias_sb[:, bias_col + m:bias_col + m + 1])
                outs.append(o)
            return outs

        def body(rep):
            # ================= embedding =================
            xT = [hp.tile([P, TOK], F32R, name="xT", tag="f1", bufs=KF)
                  for _ in range(KV)]
            for t in range(TOK // P):
                xt = sp.tile([P, VFS], F32, name="xt", tag="xt", bufs=1)
                nc.sync.dma_start(xt[:], xs[t * P:(t + 1) * P, :])
                bns = sp.tile([P, (VFS // 512) * 6], F32, name="bns",
                              tag="bns", bufs=1)
                for a in range(VFS // 512):
                    nc.vector.bn_stats(bns[:, a * 6:(a + 1) * 6],
                                       xt[:, a * 512:(a + 1) * 512])
                st2 = sp.tile([P, 2], F32, name="st2", tag="st2", bufs=1)
                nc.vector.bn_aggr(st2[:], bns[:].rearrange(
                    "p (a b) -> p a b", b=6))
                sd = sp.tile([P, 1], F32, name="xsd", tag="xsd", bufs=1)
                nc.scalar.activation(sd[:], st2[:, 1:2], AF.Sqrt, bias=eps_col[:])
                rstd = sp.tile([P, 1], F32, name="xrstd", tag="xrstd", bufs=1)
                nc.vector.reciprocal(rstd[:], sd[:])
                nmur = sp.tile([P, 1], F32, name="xnmur", tag="xnmur", bufs=1)
                nc.vector.tensor_mul(nmur[:], st2[:, 0:1], rstd[:])
                nc.scalar.activation(nmur[:], nmur[:], AF.Copy, scale=-1.0)
                xn = sp.tile([P, VFS], F32, name="xn", tag="xn", bufs=1)
                nc.scalar.activation(xn[:], xt[:], AF.Identity, bias=nmur[:],
                                     scale=rstd[:])
                for f in range(KV):
                    pt = psum("ptr")
                    nc.tensor.transpose(pt[:, 0:P], xn[:, f * P:(f + 1) * P],
                                        ident[:])
                    nc.scalar.activation(xT[f][:, t * P:(t + 1) * P],
                                         pt[:, 0:P], AF.Copy)
            if dbg and rep == 0:
                for f in range(KV):
                    nc.sync.dma_start(dbg_t["dbg_xT"][f * P:(f + 1) * P, :],
                                      xT[f][:].bitcast(F32))
            embb_sb = sp.tile([P, KD], F32, name="embb_sb", tag="embb", bufs=1)
            nc.sync.dma_start(embb_sb[:], embbc[:])
            h = proj_fm(embw[:, :], xT, embb_sb, 0, func=AF.Relu, tag="kT")
            if dbg and rep == 0:
                for f in range(KD):
                    nc.sync.dma_start(dbg_t["dbg_emb"][f * P:(f + 1) * P, :],
                                      h[f][:].bitcast(F32))
            eg_sb = sp.tile([P, 4 * KD], F32, name="eg_sb", tag="eg", bufs=1)
            nc.sync.dma_start(eg_sb[:], eg[:])
            h = ln_fm(h, eg_sb, 0 * KD, 1 * KD)
            if dbg and rep == 0:
                for f in range(KD):
                    nc.sync.dma_start(dbg_t["dbg_ln2"][f * P:(f + 1) * P, :],
                                      h[f][:].bitcast(F32))
            pos_sb = sp.tile([P, KD * TOK], F32, name="pos_sb", tag="pos",
                             bufs=1)
            for i in range(KD):
                nc.sync.dma_start(pos_sb[:, i * TOK:(i + 1) * TOK],
                                  post[i * P:(i + 1) * P, :])
            h2 = []
            for i in range(KD):
                o = hp.tile([P, TOK], F32R, name="hpos", tag="qT", bufs=KD)
                nc.vector.tensor_add(o[:], h[i][:].bitcast(F32),
                                     pos_sb[:, i * TOK:(i + 1) * TOK])
                h2.append(o)
            if dbg and rep == 0:
                for f in range(KD):
                    nc.sync.dma_start(dbg_t["dbg_pos"][f * P:(f + 1) * P, :],
                                      h2[f][:].bitcast(F32))
            h = ln_fm(h2, eg_sb, 2 * KD, 3 * KD,
                      out_dtype=F32 if n_layers == 0 else F32R)

            # ================= layers =================
            for l in range(n_layers):
                lw = 0 if inline else l
                kin, kout, vin, vo_ = ccs[rep * n_layers + l]
                bc = sp.tile([P, 8 * KD], F32, name="bc", tag="bc", bufs=2)
                nc.sync.dma_start(bc[:], bcol[lw])
                b1c_sb = sp.tile([P, KF], F32, name="b1c_sb", tag="b1c", bufs=2)
                nc.sync.dma_start(b1c_sb[:], b1col[lw])
                bv_sb = sp.tile([1, D], F32R, name="bv_sb", tag="bv", bufs=2)
                nc.sync.dma_start(bv_sb[:], bvr[lw])

                # K projection -> bounce -> AllGather
                kT = proj_fm(wk[lw], h, bc, 0, tag="kT")
                for i in range(KD):
                    nc.sync.dma_start(kin[i * P:(i + 1) * P, :], kT[i][:])
                if sim1 or "nocc" in ablate:
                    for r in range(GROUP):
                        nc.sync.dma_start(kout[r * D:(r + 1) * D, :], kin[:])
                else:
                    nc.gpsimd.collective_compute(
                        "AllGather", mybir.AluOpType.bypass,
                        ins=[kin[:].opt()], outs=[kout[:].opt()],
                        replica_groups=rg)

                # V projection (token-major) -> bounce -> AllGather
                vps = [psum(f"pp{i}") for i in range(4)]  # (t, nh) groups
                for k in range(KD):
                    st = wp.tile([P, D], F32R, name="wst", tag="w", bufs=4)
                    nc.sync.dma_start(st[:], wv[lw, k * P:(k + 1) * P, :])
                    for t in range(2):
                        for nh in range(2):
                            nc.tensor.matmul(
                                vps[t * 2 + nh][:, 0:512],
                                h[k][:, t * P:(t + 1) * P],
                                st[:, nh * 512:(nh + 1) * 512],
                                start=(k == 0), stop=False)
                for t in range(2):
                    for nh in range(2):
                        nc.tensor.matmul(
                            vps[t * 2 + nh][:, 0:512],
                            ones_row[:], bv_sb[:, nh * 512:(nh + 1) * 512],
                            start=False, stop=True)
                        vtm = sp.tile([P, 512], F32R, name="vtm", tag="vtm",
                                      bufs=2)
                        nc.scalar.activation(
                            vtm[:], vps[t * 2 + nh][:, 0:512], AF.Copy)
                        nc.sync.dma_start(
                            vin[t * P:(t + 1) * P, nh * 512:(nh + 1) * 512],
                            vtm[:])
                if sim1 or "nocc" in ablate:
                    for r in range(GROUP):
                        nc.sync.dma_start(vo_[r * TOK:(r + 1) * TOK, :], vin[:])
                else:
                    nc.gpsimd.collective_compute(
                        "AllGather", mybir.AluOpType.bypass,
                        ins=[vin[:].opt()], outs=[vo_[:].opt()],
                        replica_groups=rg)

                # Q projection (local)
                qT = proj_fm(wq[lw], h, bc, KD, tag="qT")

                # attention
                oT = [hp.tile([P, TOK], F32R, name="oT", tag="oT", bufs=KD)
                      for _ in range(KD)]
                kh1 = {}
                for hh in range(H):
                    off = (hh % 2) * DEPTH
                    kh = []
                    for r in range(GROUP):
                        if "kh1" in ablate and (off, r) in kh1:
                            kh.append(kh1[(off, r)])
                            continue
                        t_ = sp.tile([P, TOK], F32R, name="kh", tag="kh",
                                     bufs=5 if "kh1" not in ablate else 2)
                        nc.sync.dma_start(
                            t_[off:off + DEPTH, :],
                            kout[r * D + hh * DEPTH:r * D + (hh + 1) * DEPTH, :])
                        kh.append(t_)
                        kh1[(off, r)] = t_
                    qh = qT[hh // 2][off:off + DEPTH, :]
                    Es = []
                    for j in range(KD):
                        pl = psum(f"pl{j}")
                        nc.tensor.matmul(
                            pl[:, 0:TOK],
                            kh[j // 2][off:off + DEPTH,
                                       (j % 2) * P:(j % 2 + 1) * P],
                            qh, start=True, stop=True)
                        e = sp.tile([P, TOK], F32R, name="E", tag="E",
                                    bufs=9)
                        nc.scalar.activation(
                            e[:], pl[:, 0:TOK], AF.Exp, scale=SCALE,
                            bias=mask_sb[:, j:j + 1])
                        Es.append(e)
                    pso_t = psum("pso")
                    pso = pso_t[0:DEPTH + 1, 0:TOK]
                    for j in range(KD):
                        if "va1" in ablate and (hh + j) > 0:
                            va = va1_tile
                        else:
                            va = sp.tile([P, DEPTH + 1], F32R, name="va",
                                         tag="va",
                                         bufs=10 if "va1" not in ablate else 2)
                            nc.sync.dma_start(
                                va[:, 0:DEPTH],
                                vo_[j * P:(j + 1) * P,
                                    hh * DEPTH:(hh + 1) * DEPTH])
                            nc.vector.tensor_copy(va[:, DEPTH:DEPTH + 1],
                                                  ones_col[:])
                            va1_tile = va
                        nc.tensor.matmul(pso, va[:], Es[j][:],
                                         start=(j == 0), stop=(j == KD - 1))
                    r_r = sp.tile([1, TOK], F32R, name="r_r", tag="r_r",
                                  bufs=3)
                    nc.vector.reciprocal(r_r[:], pso_t[DEPTH:DEPTH + 1, 0:TOK])
                    prb = psum("prb")
                    nc.tensor.matmul(prb[0:DEPTH, 0:TOK], ones_row[:, 0:DEPTH],
                                     r_r[:], start=True, stop=True)
                    rb = sp.tile([DEPTH, TOK], F32, name="rb", tag="rb",
                                 bufs=3)
                    nc.scalar.activation(rb[:], prb[0:DEPTH, 0:TOK], AF.Copy)
                    nc.vector.tensor_mul(
                        oT[hh // 2][(hh % 2) * DEPTH:(hh % 2 + 1) * DEPTH, :],
                        pso_t[0:DEPTH, 0:TOK], rb[:])

                # output projection + residual + LN1
                aoT = proj_fm(wo[lw], oT, bc, 2 * KD, out_dtype=F32, tag="aoT")
                hr = []
                for i in range(KD):
                    t_ = hp.tile([P, TOK], F32R, name="hr", tag="hr", bufs=KD)
                    nc.vector.tensor_add(t_[:], h[i][:].bitcast(F32),
                                         aoT[i][:])
                    hr.append(t_)
                h = ln_fm(hr, bc, 4 * KD, 5 * KD)

                # FFN
                f1 = []
                for blk in range(4):
                    f1 += proj_fm(w1[lw], h, b1c_sb, blk * KD, func=AF.Prelu,
                                  alpha=0.2, tag="f1", out_bufs=KF,
                                  col0=blk * D)
                f2 = []
                f2ps = [psum(f"fp{m}") for m in range(KD)]
                for k in range(KF):
                    st = wp.tile([P, D], F32R, name="wst", tag="w", bufs=4)
                    nc.sync.dma_start(st[:], w2[lw, k * P:(k + 1) * P, :])
                    for m in range(KD):
                        nc.tensor.matmul(
                            f2ps[m][:, 0:TOK], st[:, m * P:(m + 1) * P],
                            f1[k][:], start=(k == 0), stop=(k == KF - 1))
                for m in range(KD):
                    o = hp.tile([P, TOK], F32, name="f2", tag="f2", bufs=KD)
                    nc.scalar.activation(
                        o[:], f2ps[m][:, 0:TOK], AF.Identity,
                        bias=bc[:, 3 * KD + m:3 * KD + m + 1])
                    f2.append(o)
                hr2 = []
                for i in range(KD):
                    t_ = hp.tile([P, TOK], F32R, name="hr2", tag="hr2",
                                 bufs=KD)
                    nc.vector.tensor_add(t_[:], h[i][:].bitcast(F32), f2[i][:])
                    hr2.append(t_)
                h = ln_fm(hr2, bc, 6 * KD, 7 * KD,
                          out_dtype=F32 if l == n_layers - 1 else F32R)

            # ================= output transpose =================
            for i in range(KD):
                for t in range(TOK // P):
                    pt = psum("ptr")
                    nc.tensor.transpose(pt[:, 0:P], h[i][:, t * P:(t + 1) * P],
                                        ident[:])
                    ot = sp.tile([P, P], F32, name="otile", tag="ot", bufs=3)
                    nc.scalar.activation(ot[:], pt[:, 0:P], AF.Copy)
                    nc.sync.dma_start(
                        out[t * P:(t + 1) * P, i * P:(i + 1) * P], ot[:])

        for rep in range(reps):
            body(rep)
        nc.sync.dma_start(chain_out[:], chain[:])
        stack.close()

    nc.compile()
    return nc


# ------------------------------------------------------------ host side ----

def _pos_encoding(position, d_model):
    pos = np.arange(position)[:, None].astype(np.float64)
    i = np.arange(d_model)[None, :]
    rates = 1.0 / np.power(10000, 2 * (i // 2) / np.float32(d_model))
    ang = pos * rates
    ang[:, 0::2] = np.sin(ang[:, 0::2])
    ang[:, 1::2] = np.cos(ang[:, 1::2])
    return ang.astype(np.float32)


def _cols(v):
    """[n*128] -> [128, n] (col m, partition p = v[m*128+p])."""
    return np.ascontiguousarray(np.asarray(v, np.float32).reshape(-1, P).T)


def make_in_maps(inputs):
    x = np.asarray(inputs["x"], np.float32)
    mask = np.asarray(inputs["mask"], np.float32).reshape(B, S)
    pos = _pos_encoding(MAXPOS, D)[:S]

    emb_ln1_g = np.asarray(inputs["emb_ln1_g"], np.float32)
    emb_ln1_b = np.asarray(inputs["emb_ln1_b"], np.float32)
    emb_w = np.asarray(inputs["emb_w"], np.float32)
    emb_b = np.asarray(inputs["emb_b"], np.float32)
    embw_f = emb_ln1_g[:, None] * emb_w
    embb_f = emb_b + emb_ln1_b @ emb_w

    # eg: [ln2_g | ln2_b | ln3_g | ln3_b] column tiles
    eg_np = np.concatenate([
        _cols(inputs["emb_ln2_g"]), _cols(inputs["emb_ln2_b"]),
        _cols(inputs["emb_ln3_g"]), _cols(inputs["emb_ln3_b"])], axis=1)
    # bcol per layer: [bk | bq | bo | b2 | ln1_g | ln1_b | ln2_g | ln2_b]
    bcol_np = np.stack([
        np.concatenate([
            _cols(inputs["bk"][l]), _cols(inputs["bq"][l]),
            _cols(inputs["bo"][l]), _cols(inputs["ffn_b2"][l]),
            _cols(inputs["ln1_g"][l]), _cols(inputs["ln1_b"][l]),
            _cols(inputs["ln2_g"][l]), _cols(inputs["ln2_b"][l])], axis=1)
        for l in range(L)])

    shared = {
        "embw": embw_f,
        "embbc": _cols(embb_f),
        "eg": eg_np,
        "wq": np.asarray(inputs["wq"], np.float32),
        "wk": np.asarray(inputs["wk"], np.float32),
        "wv": np.asarray(inputs["wv"], np.float32),
        "wo": np.asarray(inputs["wo"], np.float32),
        "w1": np.asarray(inputs["ffn_w1"], np.float32),
        "w2": np.asarray(inputs["ffn_w2"], np.float32),
        "bcol": bcol_np,
        "b1col": np.stack([_cols(inputs["ffn_b1"][l]) for l in range(L)]),
        "bvr": np.asarray(inputs["bv"], np.float32).reshape(L, 1, D),
        "chain": np.zeros((1, 1), np.float32),
    }
    in_maps = []
    for c in range(NCORES):
        b = c // GROUP
        t0 = (c % GROUP) * TOK
        m = dict(shared)
        m["xs"] = np.ascontiguousarray(x[b, t0:t0 + TOK, :])
        m["post"] = np.ascontiguousarray(pos[t0:t0 + TOK, :].T)
        m["maskc"] = _cols(mask[b] * (-1e9) * SCALE)
        in_maps.append(m)
    return in_maps


_CACHE = {}


def kernel(**inputs):
    if "graph" not in _CACHE:
        _CACHE["graph"] = build_graph()
    nc = _CACHE["graph"]
    in_maps = make_in_maps(inputs)
    res = run_bass_kernel_spmd(nc, in_maps, core_ids=list(range(NCORES)))
    h = np.zeros((B, S, D), np.float32)
    for c in range(NCORES):
        b = c // GROUP
        t0 = (c % GROUP) * TOK
        h[b, t0:t0 + TOK, :] = res.results[c]["out"]
    return h

